# revision 27
# baseline (speedup 1.0000x reference)
"""Self-contained Trainium2 Bass kernel for the 2-layer GCN problem.

kernel(src, dst, vals, x, W1, W2) -> [80000, 40] float32 logits,
computed as  A @ (relu((A @ x) @ W1) @ W2)  on 8 NeuronCores.

Strategy: nodes sharded round-robin across cores in 128-node slots
(graph-parallel); W1/W2 replicated.  SpMM = banked int16 dma_gather of
256B table rows (4 SWDGE queues) + per-chunk selection-matrix matmuls
accumulated in PSUM.  The cross-partition z2 exchange is 4 piecewise
AllGathers into Shared DRAM tensors, issued as phase-1 superblocks
complete so phase-3 gathers overlap the tail of phase 1.
"""
import numpy as np
import ml_dtypes
import jax
from jax.sharding import Mesh, PartitionSpec, NamedSharding
from jax.experimental.shard_map import shard_map

import concourse.bass as bass
import concourse.bacc as bacc
import concourse.tile as tile
import concourse.mybir as mybir
from concourse import bass2jax
from concourse.bass2jax import _bass_exec_p, install_neuronx_cc_hook, partition_id_tensor
from concourse.masks import make_identity

NUM_NODES = 80000
NUM_EDGES = 1280000

import os
NC, P, GRP = 8, 128, 1024
SUPER = 8
SLABMAX = int(os.environ.get("SLABMAX", 48))
G3BUFS = int(os.environ.get("G3BUFS", 4))
NPIECE = int(os.environ.get("NPIECE", 3))

F_IN = 64      # x features
F_HID = 128
F_OUT = 40
FE = 128       # bf16 elements per 256B table row

bf16 = mybir.dt.bfloat16
f32 = mybir.dt.float32


def build_stream(ec, ej, bank, lidx, vals, srel, NBLK, NB, order,
                 guard="block"):
    """Pack an edge list into the banked chunk stream.

    ec/ej: src core/block per edge; bank/lidx: gather bank + row-in-bank
    per edge; order: "Sb" (superblock outer, bank inner — phase 1) or
    "bS" (bank/piece outer — phase 3).  Returns chunk stream dict.
    """
    E = ec.shape[0]
    NSB = -(-NBLK // SUPER)
    lidx = lidx.astype(np.int16)

    key = (ec * NBLK + ej) * NB + bank
    ord_e = np.lexsort((lidx, key))
    ks = key[ord_e]
    first = np.r_[0, np.flatnonzero(np.diff(ks)) + 1]
    group_start = np.zeros(E, np.int64)
    group_start[first] = first
    group_start = np.maximum.accumulate(group_start)
    k_in_group = np.arange(E) - group_start

    cnt = np.bincount(key, minlength=NC * NBLK * NB).reshape(NC, NBLK, NB)
    Kc = -(-cnt // P)
    Kg = Kc.max(axis=0)                       # [NBLK, NB]
    if guard == "block":
        empty = Kg.sum(axis=1) == 0
        Kg[empty, 0] = 1
    else:                                     # per-cell guard (phase 3)
        Kg = np.maximum(Kg, 1)

    Kmax = int(Kg.max())
    # chunk_map[j, b, k] -> chunk id of the k-th chunk of cell (j, b)
    chunk_map = np.full((NBLK, NB, Kmax), -1, np.int64)
    slabs = []            # (start_chunk, n_chunks, bank, group_id)
    chunk_block = []
    gb_first = {}
    gb_last = {}
    pos = 0

    def emit_run(js, b, gid):
        """Emit cell (j in js, bank b) chunks interleaved round-robin over j
        so consecutive matmuls hit different PSUM regions."""
        nonlocal pos
        run_start = pos
        items = [(k, j) for j in js for k in range(Kg[j, b])]
        items.sort()
        for (k, j) in items:
            chunk_map[j, b, k] = pos
            chunk_block.append(j)
            pos += 1
        run_len = pos - run_start
        o = run_start
        while run_len > 0:
            take = min(run_len, SLABMAX)
            slabs.append((o, take, b, gid))
            o += take
            run_len -= take

    if order == "Sb":
        for S in range(NSB):
            js = list(range(S * SUPER, min((S + 1) * SUPER, NBLK)))
            gid = S
            g0 = pos
            for b in range(NB):
                emit_run(js, b, gid)
            gb_first[gid] = g0
            gb_last[gid] = pos - 1
    else:  # "bS": bank (piece) outer, superblock inner
        for b in range(NB):
            for S in range(NSB):
                js = list(range(S * SUPER, min((S + 1) * SUPER, NBLK)))
                gid = b * NSB + S
                g0 = pos
                emit_run(js, b, gid)
                gb_first[gid] = g0
                gb_last[gid] = pos - 1
    NCHUNK = pos
    chunk_block = np.asarray(chunk_block)

    gidx = np.zeros((NC, NCHUNK * P), np.int16)
    gval = np.zeros((NC, NCHUNK * P), np.float32)
    gsrel = np.zeros((NC, NCHUNK * P), np.float32)
    echunk = chunk_map[ej[ord_e], bank[ord_e], k_in_group // P]
    assert (echunk >= 0).all()
    epos = echunk * P + (k_in_group % P)
    core_o = ec[ord_e]
    gidx[core_o, epos] = lidx[ord_e]
    gval[core_o, epos] = vals[ord_e]
    gsrel[core_o, epos] = srel[ord_e]

    return dict(NCHUNK=NCHUNK, slabs=slabs, chunk_block=chunk_block,
                gb_first=gb_first, gb_last=gb_last, Kg=Kg,
                gidx=gidx, gval=gval, gsrel=gsrel)


def build_layout(src, dst, vals, n_nodes, banksz=32768):
    NBLK = -(-n_nodes // GRP)
    TAB = NC * NBLK * P
    NB1 = -(-TAB // banksz)
    NSB = -(-NBLK // SUPER)

    n = np.arange(n_nodes)
    c_of = (n // P) % NC
    j_of = n // GRP
    s_of = n % P
    table_row = (c_of * NBLK + j_of) * P + s_of

    ec, ej, es = c_of[src], j_of[src], s_of[src]
    srel = es.astype(np.float32)

    # phase 1: dst -> xtab row / bank
    r1 = table_row[dst]
    b1 = r1 // banksz
    l1 = r1 % banksz
    st1 = build_stream(ec, ej, b1, l1, vals, srel, NBLK, NB1, "Sb",
                       guard="block")

    # phase 3: dst -> piece (block range of its owner), piece-local row
    pb = -(-NBLK // NPIECE)
    plens = [min((p + 1) * pb, NBLK) - p * pb for p in range(NPIECE)]
    pjd = j_of[dst]
    pcd = c_of[dst]
    psd_ = s_of[dst]
    p3 = pjd // pb
    assert NC * pb * P <= 32768, "piece rows exceed int16 gather index range"
    l3 = (pcd * np.asarray(plens)[p3] + (pjd - p3 * pb)) * P + psd_
    st3 = build_stream(ec, ej, p3, l3, vals, srel, NBLK, NPIECE, "bS",
                       guard="cell")

    return dict(NBLK=NBLK, TAB=TAB, NB1=NB1, NSB=NSB, banksz=banksz,
                pb=pb, plens=plens, table_row=table_row, st1=st1, st3=st3)


def wrap_cols(a, NCHUNK):
    """[NC, NCHUNK*128] -> per-core [128, NCHUNK*8] wrapped int16 tiles."""
    out = []
    for c in range(NC):
        n = a.shape[1]
        w = a[c].reshape(n // 16, 16).T
        out.append(np.tile(w, (8, 1)).copy())
    return out


def build_nc(L, shared_out=True, only_phase=None, ablate=None):
    NBLK, TAB, NB1, NSB = L["NBLK"], L["TAB"], L["NB1"], L["NSB"]
    banksz = L["banksz"]
    pb, plens = L["pb"], L["plens"]
    st1, st3 = L["st1"], L["st3"]
    NCH1, NCH3 = st1["NCHUNK"], st3["NCHUNK"]
    maxslab = max(s[1] for s in st1["slabs"] + st3["slabs"])

    nc = bacc.Bacc("TRN2", target_bir_lowering=False, debug=False,
                   num_devices=NC, num_swdge_queues=4)
    xtab = nc.dram_tensor("xtab", [TAB, FE], bf16, kind="ExternalInput")
    z2in = None
    if only_phase == 3:
        z2in = nc.dram_tensor("z2in", [NBLK * P, FE], bf16, kind="ExternalInput")
    gidx1 = nc.dram_tensor("gidx1", [P, NCH1 * 8], mybir.dt.int16, kind="ExternalInput")
    gval1 = nc.dram_tensor("gval1", [P, NCH1], f32, kind="ExternalInput")
    gsrel1 = nc.dram_tensor("gsrel1", [P, NCH1], f32, kind="ExternalInput")
    gidx3 = nc.dram_tensor("gidx3", [P, NCH3 * 8], mybir.dt.int16, kind="ExternalInput")
    gval3 = nc.dram_tensor("gval3", [P, NCH3], f32, kind="ExternalInput")
    gsrel3 = nc.dram_tensor("gsrel3", [P, NCH3], f32, kind="ExternalInput")
    w1 = nc.dram_tensor("w1", [F_IN, F_HID], bf16, kind="ExternalInput")
    w2 = nc.dram_tensor("w2", [F_HID, 64], bf16, kind="ExternalInput")
    iota_in = nc.dram_tensor("iota", [P, P], bf16, kind="ExternalInput")
    out_ext = nc.dram_tensor("out", [NBLK * P, F_OUT], f32, kind="ExternalOutput")

    def bank_rows(b):
        return slice(b * banksz, min((b + 1) * banksz, TAB))

    with tile.TileContext(nc) as tc:
        with (
            tc.tile_pool(name="cons", bufs=1) as cons,
            tc.tile_pool(name="sbuf", bufs=G3BUFS) as sbuf,
            tc.tile_pool(name="sv", bufs=32) as svp,
            tc.tile_pool(name="dense", bufs=2) as dns,
            tc.tile_pool(name="psum", bufs=2, space="PSUM") as psum,
            tc.tile_pool(name="psd", bufs=2, space="PSUM") as psd,
            tc.tile_pool(name="dram", bufs=1, space="DRAM") as dram,
        ):
            iota_t = cons.tile([P, P], bf16)
            ident_t = cons.tile([P, P], bf16)
            make_identity(nc, ident_t[:])
            w1_t = cons.tile([F_IN, F_HID], bf16)
            w2_t = cons.tile([F_HID, 64], bf16)
            idx1_t = cons.tile([P, NCH1 * 8], mybir.dt.int16)
            val1_t = cons.tile([P, NCH1], f32)
            srel1_t = cons.tile([P, NCH1], f32)
            idx3_t = cons.tile([P, NCH3 * 8], mybir.dt.int16)
            val3_t = cons.tile([P, NCH3], f32)
            srel3_t = cons.tile([P, NCH3], f32)
            outacc = cons.tile([P, NBLK * F_OUT], f32)
            nc.sync.dma_start(out=iota_t[:], in_=iota_in[:, :])
            nc.sync.dma_start(out=w1_t[:], in_=w1[:, :])
            nc.sync.dma_start(out=w2_t[:], in_=w2[:, :])
            nc.sync.dma_start(out=idx1_t[:], in_=gidx1[:, :])
            nc.sync.dma_start(out=val1_t[:], in_=gval1[:, :])
            nc.sync.dma_start(out=srel1_t[:], in_=gsrel1[:, :])
            nc.sync.dma_start(out=idx3_t[:], in_=gidx3[:, :])
            nc.sync.dma_start(out=val3_t[:], in_=gval3[:, :])
            nc.sync.dma_start(out=srel3_t[:], in_=gsrel3[:, :])

            z2locp = [dram.tile([plens[p] * P, FE], bf16, name=f"z2locp{p}")
                      for p in range(NPIECE)]
            z2p = [dram.tile([NC * plens[p] * P, FE], bf16, name=f"z2p{p}",
                             addr_space=("Shared" if shared_out else "Local"))
                   for p in range(NPIECE)]

            qctr = [0]
            sv_hoist = None
            if ablate == "svhoist":
                sv_hoist = cons.tile([P, P], bf16)
                nc.vector.tensor_scalar(
                    out=sv_hoist[:], in0=iota_t[:],
                    scalar1=srel1_t[:, 0:1], scalar2=val1_t[:, 0:1],
                    op0=mybir.AluOpType.is_equal, op1=mybir.AluOpType.mult)

            def do_slabs(slabs_sel, st, idx_t, val_t, srel_t, table_of,
                         fcols, acc_of, gtag):
                """Run gather+selection-matmul for the given slab list."""
                for (c0, Ln, b, gid) in slabs_sel:
                    g3 = sbuf.tile([P, maxslab, FE], bf16, tag=gtag)
                    nc.gpsimd.dma_gather(
                        g3[:, 0:Ln, :],
                        table_of(b),
                        idx_t[:, c0 * 8:(c0 + Ln) * 8],
                        Ln * P,
                        Ln * P,
                        FE,
                        single_packet=False,
                        queue_num=(qctr[0] % 4),
                    )
                    qctr[0] += 1
                    if ablate == "gonly":
                        continue
                    for t in range(Ln):
                        ch = c0 + t
                        j = int(st["chunk_block"][ch])
                        jj = j % SUPER
                        if ablate == "svhoist":
                            sv = sv_hoist
                        else:
                            sv = svp.tile([P, P], bf16, tag="sv")
                            nc.vector.tensor_scalar(
                                out=sv[:], in0=iota_t[:],
                                scalar1=srel_t[:, ch:ch + 1],
                                scalar2=val_t[:, ch:ch + 1],
                                op0=mybir.AluOpType.is_equal,
                                op1=mybir.AluOpType.mult,
                            )
                        if ablate == "nope":
                            continue
                        acc = acc_of(gid)
                        nc.tensor.matmul(
                            out=acc[:, 64 * jj:64 * jj + fcols],
                            lhsT=sv[:],
                            rhs=g3[:, t, 0:fcols],
                            start=(ch == st["gb_first"][gid]),
                            stop=(ch == st["gb_last"][gid]),
                            skip_group_check=True,
                        )

            # ---- phase 1: z1 = A@x ; dense chain ; z2 piece shards ----
            ag_issued = [False] * NPIECE

            def piece_of_block(j):
                return j // pb

            def phase1_block(j, acc_ap):
                z1_sb = dns.tile([P, F_IN], bf16, tag="z1")
                nc.vector.tensor_copy(out=z1_sb[:], in_=acc_ap)
                pt = psd.tile([F_IN, P], bf16, tag="pt")
                nc.tensor.transpose(out=pt[:], in_=z1_sb[:], identity=ident_t[:])
                z1t = dns.tile([F_IN, P], bf16, tag="z1t")
                nc.vector.tensor_copy(out=z1t[:], in_=pt[:])
                ph = psd.tile([F_HID, P], f32, tag="pd")
                nc.tensor.matmul(out=ph[:], lhsT=w1_t[:], rhs=z1t[:],
                                 start=True, stop=True)
                ht = dns.tile([F_HID, P], bf16, tag="ht")
                nc.scalar.activation(out=ht[:], in_=ph[:],
                                     func=mybir.ActivationFunctionType.Relu)
                pz = psd.tile([P, 64], f32, tag="pd")
                nc.tensor.matmul(out=pz[:], lhsT=ht[:], rhs=w2_t[:],
                                 start=True, stop=True)
                z2_sb = dns.tile([P, 64], bf16, tag="z2")
                nc.scalar.copy(out=z2_sb[:], in_=pz[:])
                p = piece_of_block(j)
                jl = j - p * pb
                nc.sync.dma_start(
                    out=z2locp[p][jl * P:(jl + 1) * P, 0:64], in_=z2_sb[:])
                if only_phase == 1:
                    o1 = dns.tile([P, 64], f32, tag="o1")
                    nc.vector.tensor_copy(out=o1[:], in_=pz[:])
                    nc.sync.dma_start(
                        out=out_ext[j * P:(j + 1) * P, :], in_=o1[:, 0:F_OUT])

            def issue_ag(p):
                out3 = z2p[p][:].rearrange("(c r) f -> c r f", c=NC)
                nc.gpsimd.collective_compute(
                    "AllGather",
                    mybir.AluOpType.bypass,
                    replica_groups=[list(range(NC))],
                    ins=[z2locp[p][:].opt()],
                    outs=[out3[:, :, :].opt()],
                )

            slabs1 = st1["slabs"]
            slabs3 = st3["slabs"]
            acc1 = {}
            acc3 = {}

            def emit_ph1_sb(S):
                jlo, jhi = S * SUPER, min((S + 1) * SUPER, NBLK)
                acc_t = psum.tile([P, 64 * (jhi - jlo)], f32, tag="acc")
                acc1[S] = acc_t
                do_slabs([s for s in slabs1 if s[3] == S], st1,
                         idx1_t, val1_t, srel1_t,
                         lambda b: xtab[bank_rows(b), :], F_IN,
                         lambda gid: acc1[gid], "g1")
                if ablate in ("gonly", "nope"):
                    return
                if ablate in ("nodense", "svhoist"):
                    dr = dns.tile([P, 64 * (jhi - jlo)], bf16, tag="dr")
                    nc.scalar.copy(out=dr[:], in_=acc_t[:])
                    return
                for j in range(jlo, jhi):
                    jj = j - jlo
                    phase1_block(j, acc_t[:, 64 * jj:64 * jj + F_IN])

            def emit_ph3_piece(pp):
                for S in range(NSB):
                    gid = pp * NSB + S
                    jlo, jhi = S * SUPER, min((S + 1) * SUPER, NBLK)
                    acc_t = psum.tile([P, 64 * (jhi - jlo)], f32, tag="acc")
                    acc3[gid] = acc_t
                    do_slabs([s for s in slabs3 if s[3] == gid], st3,
                             idx3_t, val3_t, srel3_t,
                             lambda b: z2p[b][:, :], F_OUT,
                             lambda g: acc3[g], "g2")
                    for j in range(jlo, jhi):
                        jj = j - jlo
                        src = acc_t[:, 64 * jj:64 * jj + F_OUT]
                        dsts = outacc[:, j * F_OUT:(j + 1) * F_OUT]
                        if pp == 0:
                            nc.scalar.copy(out=dsts, in_=src)
                        else:
                            nc.vector.tensor_tensor(
                                out=dsts, in0=dsts, in1=src,
                                op=mybir.AluOpType.add)

            if only_phase == 3:
                for p in range(NPIECE):
                    nc.sync.dma_start(
                        out=z2locp[p][:, :],
                        in_=z2in[p * pb * P:(p * pb + plens[p]) * P, :])
                    issue_ag(p)
                for pp in range(NPIECE):
                    emit_ph3_piece(pp)
            elif only_phase == 1 or ablate is not None:
                for S in range(NSB):
                    emit_ph1_sb(S)
                    if ablate is None:
                        for p in range(NPIECE):
                            if not ag_issued[p] and \
                               min((S + 1) * SUPER, NBLK) >= min((p + 1) * pb, NBLK):
                                ag_issued[p] = True
                                issue_ag(p)
                if ablate is not None:
                    nc.vector.memset(outacc[:, 0:F_OUT], 0.0)
                    nc.sync.dma_start(out=out_ext[0:P, :],
                                      in_=outacc[:, 0:F_OUT])
            else:
                # full: interleave — AGs issued one SB after data-ready;
                # ph3 pieces emitted once their AG has had time to land.
                def ready_sb(p):
                    tgt = min((p + 1) * pb, NBLK)
                    for S in range(NSB):
                        if min((S + 1) * SUPER, NBLK) >= tgt:
                            return S
                    return NSB - 1

                ag_after = {}
                ph3_after = {}
                for p in range(NPIECE):
                    ag_after.setdefault(min(ready_sb(p) + 1, NSB - 1), []).append(p)
                    ph3_after.setdefault(min(ready_sb(p) + 4, NSB - 1), []).append(p)
                for S in range(NSB):
                    emit_ph1_sb(S)
                    for p in ag_after.get(S, []):
                        issue_ag(p)
                    for pp in ph3_after.get(S, []):
                        emit_ph3_piece(pp)

            if only_phase != 1 and ablate is None:
                # final output DMA (one per block)
                oview = out_ext[:].rearrange("(j s) f -> s j f", s=P)
                nc.sync.dma_start(
                    out=oview[:, :, :],
                    in_=outacc[:].rearrange("s (j f) -> s j f", f=F_OUT))

    nc.compile()
    return nc


def pack_inputs(L, x, W1, W2):
    """Returns per-core in_maps list."""
    TAB = L["TAB"]
    st1, st3 = L["st1"], L["st3"]
    xtab = np.zeros((TAB, FE), ml_dtypes.bfloat16)
    xtab[L["table_row"], 0:F_IN] = x.astype(ml_dtypes.bfloat16)
    w1b = W1.astype(ml_dtypes.bfloat16)
    w2b = np.zeros((F_HID, 64), ml_dtypes.bfloat16)
    w2b[:, 0:F_OUT] = W2.astype(ml_dtypes.bfloat16)
    iota = np.tile(np.arange(P, dtype=np.float32), (P, 1)).astype(ml_dtypes.bfloat16)

    idx1 = wrap_cols(st1["gidx"], st1["NCHUNK"])
    idx3 = wrap_cols(st3["gidx"], st3["NCHUNK"])

    in_maps = []
    for c in range(NC):
        m = {
            "xtab": xtab,
            "gidx1": idx1[c],
            "gval1": st1["gval"][c].reshape(st1["NCHUNK"], P).T.copy(),
            "gsrel1": st1["gsrel"][c].reshape(st1["NCHUNK"], P).T.copy(),
            "gidx3": idx3[c],
            "gval3": st3["gval"][c].reshape(st3["NCHUNK"], P).T.copy(),
            "gsrel3": st3["gsrel"][c].reshape(st3["NCHUNK"], P).T.copy(),
            "w1": w1b, "w2": w2b, "iota": iota,
        }
        in_maps.append(m)
    return in_maps


def unpack_output(L, results):
    """results: list of per-core dicts with 'out' [NBLK*128, 40]."""
    outcat = np.concatenate([r["out"] for r in results], axis=0)  # [TAB, 40]
    return outcat[L["table_row"]]


def make_runner(nc, n_cores=8, donate=False):
    install_neuronx_cc_hook()
    partition_name = nc.partition_id_tensor.name if nc.partition_id_tensor else None

    in_names, out_names, out_avals, zero_outs = [], [], [], []
    for alloc in nc.m.functions[0].allocations:
        if not isinstance(alloc, mybir.MemoryLocationSet):
            continue
        name = alloc.memorylocations[0].name
        if alloc.kind == "ExternalInput":
            if name != partition_name:
                in_names.append(name)
        elif alloc.kind == "ExternalOutput":
            out_names.append(name)
            shape = tuple(alloc.tensor_shape)
            dtype = mybir.dt.np(alloc.dtype)
            out_avals.append(jax.core.ShapedArray(shape, dtype))
            zero_outs.append(np.zeros(shape, dtype))
    n_params = len(in_names)
    n_outs = len(out_avals)
    all_in_names = list(in_names) + list(out_names)
    if partition_name is not None:
        all_in_names.append(partition_name)

    def _body(*args):
        operands = list(args)
        if partition_name is not None:
            operands.append(partition_id_tensor())
        outs = _bass_exec_p.bind(
            *operands,
            out_avals=tuple(out_avals),
            in_names=tuple(all_in_names),
            out_names=tuple(out_names),
            lowering_input_output_aliases=(),
            sim_require_finite=True,
            sim_require_nnan=True,
            nc=nc,
        )
        return tuple(outs)

    devices = jax.devices()[:n_cores]
    mesh = Mesh(np.asarray(devices), ("core",))
    in_specs = (PartitionSpec("core"),) * (n_params + n_outs)
    out_specs = (PartitionSpec("core"),) * n_outs
    jit_kwargs = {"keep_unused": True}
    if donate:
        jit_kwargs["donate_argnums"] = tuple(range(n_params, n_params + n_outs))
    fn = jax.jit(
        shard_map(_body, mesh=mesh, in_specs=in_specs, out_specs=out_specs,
                  check_rep=False),
        **jit_kwargs,
    )
    sharding = NamedSharding(mesh, PartitionSpec("core"))

    class Runner:
        def __init__(self):
            self.fn = fn
            self.in_names = in_names
            self.out_names = out_names
            self.n_cores = n_cores
            self.sharding = sharding
            self.zero_outs = zero_outs

        def put_inputs(self, in_maps):
            args = []
            for name in in_names:
                cat = np.concatenate([np.asarray(m[name]) for m in in_maps], axis=0)
                args.append(jax.device_put(cat, sharding))
            for z in zero_outs:
                cat = np.concatenate([z] * n_cores, axis=0)
                args.append(jax.device_put(cat, sharding))
            return args

        def __call__(self, args):
            return self.fn(*args)

        def run(self, in_maps):
            args = self.put_inputs(in_maps)
            outs = self.fn(*args)
            jax.block_until_ready(outs)
            res = []
            for c in range(n_cores):
                d = {}
                for i, name in enumerate(out_names):
                    arr = np.asarray(outs[i])
                    per = arr.shape[0] // n_cores
                    d[name] = arr[c * per:(c + 1) * per]
                res.append(d)
            return res

    return Runner()


_CACHE = {}


def kernel(src, dst, vals, x, W1, W2):
    src = np.asarray(src); dst = np.asarray(dst)
    vals = np.asarray(vals, dtype=np.float32)
    x = np.asarray(x, dtype=np.float32)
    W1 = np.asarray(W1, dtype=np.float32)
    W2 = np.asarray(W2, dtype=np.float32)

    L = build_layout(src.astype(np.int64), dst.astype(np.int64), vals, NUM_NODES)
    key = "r"
    if key not in _CACHE:
        nc = build_nc(L)
        _CACHE[key] = make_runner(nc)
    r = _CACHE[key]
    in_maps = pack_inputs(L, x, W1, W2)
    results = r.run(in_maps)
    return unpack_output(L, results).astype(np.float32)


# revision 28
# speedup vs baseline: 1.1797x; 1.1797x over previous
"""Self-contained Trainium2 Bass kernel for the 2-layer GCN problem.

kernel(src, dst, vals, x, W1, W2) -> [80000, 40] float32 logits,
computed as  A @ (relu((A @ x) @ W1) @ W2)  on 8 NeuronCores.

Strategy: nodes sharded round-robin across cores in 128-node slots
(graph-parallel); W1/W2 replicated.  SpMM = banked int16 dma_gather of
256B table rows (4 SWDGE queues) + per-chunk selection-matrix matmuls
accumulated in PSUM.  The cross-partition z2 exchange is 4 piecewise
AllGathers into Shared DRAM tensors, issued as phase-1 superblocks
complete so phase-3 gathers overlap the tail of phase 1.
"""
import numpy as np
import ml_dtypes
import jax
from jax.sharding import Mesh, PartitionSpec, NamedSharding
from jax.experimental.shard_map import shard_map

import concourse.bass as bass
import concourse.bacc as bacc
import concourse.tile as tile
import concourse.mybir as mybir
from concourse import bass2jax
from concourse.bass2jax import _bass_exec_p, install_neuronx_cc_hook, partition_id_tensor
from concourse.masks import make_identity

NUM_NODES = 80000
NUM_EDGES = 1280000

import os
NC, P, GRP = 8, 128, 1024
SUPER = 8
SLABMAX = int(os.environ.get("SLABMAX", 48))
G3BUFS = int(os.environ.get("G3BUFS", 4))
NPIECE = int(os.environ.get("NPIECE", 3))

F_IN = 64      # x features
F_HID = 128
F_OUT = 40
FE = 128       # bf16 elements per 256B table row

bf16 = mybir.dt.bfloat16
f32 = mybir.dt.float32


def build_stream(ec, ej, bank, lidx, vals, srel, NBLK, NB, order,
                 guard="block"):
    """Pack an edge list into the banked chunk stream.

    ec/ej: src core/block per edge; bank/lidx: gather bank + row-in-bank
    per edge; order: "Sb" (superblock outer, bank inner — phase 1) or
    "bS" (bank/piece outer — phase 3).  Returns chunk stream dict.
    """
    E = ec.shape[0]
    NSB = -(-NBLK // SUPER)
    lidx = lidx.astype(np.int16)

    key = (ec * NBLK + ej) * NB + bank
    ord_e = np.lexsort((lidx, key))
    ks = key[ord_e]
    first = np.r_[0, np.flatnonzero(np.diff(ks)) + 1]
    group_start = np.zeros(E, np.int64)
    group_start[first] = first
    group_start = np.maximum.accumulate(group_start)
    k_in_group = np.arange(E) - group_start

    cnt = np.bincount(key, minlength=NC * NBLK * NB).reshape(NC, NBLK, NB)
    Kc = -(-cnt // P)
    Kg = Kc.max(axis=0)                       # [NBLK, NB]
    if guard == "block":
        empty = Kg.sum(axis=1) == 0
        Kg[empty, 0] = 1
    else:                                     # per-cell guard (phase 3)
        Kg = np.maximum(Kg, 1)

    Kmax = int(Kg.max())
    # chunk_map[j, b, k] -> chunk id of the k-th chunk of cell (j, b)
    chunk_map = np.full((NBLK, NB, Kmax), -1, np.int64)
    slabs = []            # (start_chunk, n_chunks, bank, group_id)
    chunk_block = []
    gb_first = {}
    gb_last = {}
    pos = 0

    def emit_run(js, b, gid):
        """Emit cell (j in js, bank b) chunks interleaved round-robin over j
        so consecutive matmuls hit different PSUM regions."""
        nonlocal pos
        run_start = pos
        items = [(k, j) for j in js for k in range(Kg[j, b])]
        items.sort()
        for (k, j) in items:
            chunk_map[j, b, k] = pos
            chunk_block.append(j)
            pos += 1
        run_len = pos - run_start
        o = run_start
        while run_len > 0:
            take = min(run_len, SLABMAX)
            slabs.append((o, take, b, gid))
            o += take
            run_len -= take

    if order == "Sb":
        for S in range(NSB):
            js = list(range(S * SUPER, min((S + 1) * SUPER, NBLK)))
            gid = S
            g0 = pos
            for b in range(NB):
                emit_run(js, b, gid)
            gb_first[gid] = g0
            gb_last[gid] = pos - 1
    else:  # "bS": bank (piece) outer, superblock inner
        for b in range(NB):
            for S in range(NSB):
                js = list(range(S * SUPER, min((S + 1) * SUPER, NBLK)))
                gid = b * NSB + S
                g0 = pos
                emit_run(js, b, gid)
                gb_first[gid] = g0
                gb_last[gid] = pos - 1
    NCHUNK = pos
    chunk_block = np.asarray(chunk_block)

    gidx = np.zeros((NC, NCHUNK * P), np.int16)
    gval = np.zeros((NC, NCHUNK * P), np.float32)
    gsrel = np.zeros((NC, NCHUNK * P), np.float32)
    echunk = chunk_map[ej[ord_e], bank[ord_e], k_in_group // P]
    assert (echunk >= 0).all()
    epos = echunk * P + (k_in_group % P)
    core_o = ec[ord_e]
    gidx[core_o, epos] = lidx[ord_e]
    gval[core_o, epos] = vals[ord_e]
    gsrel[core_o, epos] = srel[ord_e]

    return dict(NCHUNK=NCHUNK, slabs=slabs, chunk_block=chunk_block,
                gb_first=gb_first, gb_last=gb_last, Kg=Kg,
                gidx=gidx, gval=gval, gsrel=gsrel)


def build_layout(src, dst, vals, n_nodes, banksz=32768):
    NBLK = -(-n_nodes // GRP)
    TAB = NC * NBLK * P
    NB1 = -(-TAB // banksz)
    NSB = -(-NBLK // SUPER)

    # LPT balance: assign nodes to (core, block) groups by out-degree so
    # per-group edge totals (and hence per-core chunk counts) equalize.
    import heapq
    deg = np.bincount(src, minlength=n_nodes)
    order = np.argsort(-deg, kind="stable")
    ngroups = NC * NBLK
    cap = n_nodes - (ngroups - 1) * P  # last-filled groups may be partial
    heap = [(0, g) for g in range(ngroups)]
    heapq.heapify(heap)
    counts = np.zeros(ngroups, np.int64)
    c_of = np.empty(n_nodes, np.int64)
    j_of = np.empty(n_nodes, np.int64)
    s_of = np.empty(n_nodes, np.int64)
    for node in order:
        load, g = heapq.heappop(heap)
        c_of[node] = g % NC
        j_of[node] = g // NC
        s_of[node] = counts[g]
        counts[g] += 1
        if counts[g] < P:
            heapq.heappush(heap, (load + int(deg[node]), g))
    table_row = (c_of * NBLK + j_of) * P + s_of

    ec, ej, es = c_of[src], j_of[src], s_of[src]
    srel = es.astype(np.float32)

    # phase 1: dst -> xtab row / bank
    r1 = table_row[dst]
    b1 = r1 // banksz
    l1 = r1 % banksz
    st1 = build_stream(ec, ej, b1, l1, vals, srel, NBLK, NB1, "Sb",
                       guard="block")

    # phase 3: dst -> piece (block range of its owner), piece-local row
    pb = -(-NBLK // NPIECE)
    plens = [min((p + 1) * pb, NBLK) - p * pb for p in range(NPIECE)]
    pjd = j_of[dst]
    pcd = c_of[dst]
    psd_ = s_of[dst]
    p3 = pjd // pb
    assert NC * pb * P <= 32768, "piece rows exceed int16 gather index range"
    l3 = (pcd * np.asarray(plens)[p3] + (pjd - p3 * pb)) * P + psd_
    st3 = build_stream(ec, ej, p3, l3, vals, srel, NBLK, NPIECE, "bS",
                       guard="cell")

    return dict(NBLK=NBLK, TAB=TAB, NB1=NB1, NSB=NSB, banksz=banksz,
                pb=pb, plens=plens, table_row=table_row, st1=st1, st3=st3)


def wrap_cols(a, NCHUNK):
    """[NC, NCHUNK*128] -> per-core [128, NCHUNK*8] wrapped int16 tiles."""
    out = []
    for c in range(NC):
        n = a.shape[1]
        w = a[c].reshape(n // 16, 16).T
        out.append(np.tile(w, (8, 1)).copy())
    return out


def build_nc(L, shared_out=True, only_phase=None, ablate=None):
    NBLK, TAB, NB1, NSB = L["NBLK"], L["TAB"], L["NB1"], L["NSB"]
    banksz = L["banksz"]
    pb, plens = L["pb"], L["plens"]
    st1, st3 = L["st1"], L["st3"]
    NCH1, NCH3 = st1["NCHUNK"], st3["NCHUNK"]
    maxslab = max(s[1] for s in st1["slabs"] + st3["slabs"])

    nc = bacc.Bacc("TRN2", target_bir_lowering=False, debug=False,
                   num_devices=NC, num_swdge_queues=4)
    xtab = nc.dram_tensor("xtab", [TAB, FE], bf16, kind="ExternalInput")
    z2in = None
    if only_phase == 3:
        z2in = nc.dram_tensor("z2in", [NBLK * P, FE], bf16, kind="ExternalInput")
    gidx1 = nc.dram_tensor("gidx1", [P, NCH1 * 8], mybir.dt.int16, kind="ExternalInput")
    gval1 = nc.dram_tensor("gval1", [P, NCH1], f32, kind="ExternalInput")
    gsrel1 = nc.dram_tensor("gsrel1", [P, NCH1], f32, kind="ExternalInput")
    gidx3 = nc.dram_tensor("gidx3", [P, NCH3 * 8], mybir.dt.int16, kind="ExternalInput")
    gval3 = nc.dram_tensor("gval3", [P, NCH3], f32, kind="ExternalInput")
    gsrel3 = nc.dram_tensor("gsrel3", [P, NCH3], f32, kind="ExternalInput")
    w1 = nc.dram_tensor("w1", [F_IN, F_HID], bf16, kind="ExternalInput")
    w2 = nc.dram_tensor("w2", [F_HID, 64], bf16, kind="ExternalInput")
    iota_in = nc.dram_tensor("iota", [P, P], bf16, kind="ExternalInput")
    out_ext = nc.dram_tensor("out", [NBLK * P, F_OUT], f32, kind="ExternalOutput")

    def bank_rows(b):
        return slice(b * banksz, min((b + 1) * banksz, TAB))

    with tile.TileContext(nc) as tc:
        with (
            tc.tile_pool(name="cons", bufs=1) as cons,
            tc.tile_pool(name="sbuf", bufs=G3BUFS) as sbuf,
            tc.tile_pool(name="sv", bufs=32) as svp,
            tc.tile_pool(name="dense", bufs=2) as dns,
            tc.tile_pool(name="psum", bufs=2, space="PSUM") as psum,
            tc.tile_pool(name="psd", bufs=2, space="PSUM") as psd,
            tc.tile_pool(name="dram", bufs=1, space="DRAM") as dram,
        ):
            iota_t = cons.tile([P, P], bf16)
            ident_t = cons.tile([P, P], bf16)
            make_identity(nc, ident_t[:])
            w1_t = cons.tile([F_IN, F_HID], bf16)
            w2_t = cons.tile([F_HID, 64], bf16)
            idx1_t = cons.tile([P, NCH1 * 8], mybir.dt.int16)
            val1_t = cons.tile([P, NCH1], f32)
            srel1_t = cons.tile([P, NCH1], f32)
            idx3_t = cons.tile([P, NCH3 * 8], mybir.dt.int16)
            val3_t = cons.tile([P, NCH3], f32)
            srel3_t = cons.tile([P, NCH3], f32)
            outacc = cons.tile([P, NBLK * F_OUT], f32)
            nc.sync.dma_start(out=iota_t[:], in_=iota_in[:, :])
            nc.sync.dma_start(out=w1_t[:], in_=w1[:, :])
            nc.sync.dma_start(out=w2_t[:], in_=w2[:, :])
            nc.sync.dma_start(out=idx1_t[:], in_=gidx1[:, :])
            nc.sync.dma_start(out=val1_t[:], in_=gval1[:, :])
            nc.sync.dma_start(out=srel1_t[:], in_=gsrel1[:, :])
            nc.sync.dma_start(out=idx3_t[:], in_=gidx3[:, :])
            nc.sync.dma_start(out=val3_t[:], in_=gval3[:, :])
            nc.sync.dma_start(out=srel3_t[:], in_=gsrel3[:, :])

            z2locp = [dram.tile([plens[p] * P, FE], bf16, name=f"z2locp{p}")
                      for p in range(NPIECE)]
            z2p = [dram.tile([NC * plens[p] * P, FE], bf16, name=f"z2p{p}",
                             addr_space=("Shared" if shared_out else "Local"))
                   for p in range(NPIECE)]

            qctr = [0]
            sv_hoist = None
            if ablate == "svhoist":
                sv_hoist = cons.tile([P, P], bf16)
                nc.vector.tensor_scalar(
                    out=sv_hoist[:], in0=iota_t[:],
                    scalar1=srel1_t[:, 0:1], scalar2=val1_t[:, 0:1],
                    op0=mybir.AluOpType.is_equal, op1=mybir.AluOpType.mult)

            def do_slabs(slabs_sel, st, idx_t, val_t, srel_t, table_of,
                         fcols, acc_of, gtag):
                """Run gather+selection-matmul for the given slab list."""
                for (c0, Ln, b, gid) in slabs_sel:
                    g3 = sbuf.tile([P, maxslab, FE], bf16, tag=gtag)
                    nc.gpsimd.dma_gather(
                        g3[:, 0:Ln, :],
                        table_of(b),
                        idx_t[:, c0 * 8:(c0 + Ln) * 8],
                        Ln * P,
                        Ln * P,
                        FE,
                        single_packet=False,
                        queue_num=(qctr[0] % 4),
                    )
                    qctr[0] += 1
                    if ablate == "gonly":
                        continue
                    for t in range(Ln):
                        ch = c0 + t
                        j = int(st["chunk_block"][ch])
                        jj = j % SUPER
                        if ablate == "svhoist":
                            sv = sv_hoist
                        else:
                            sv = svp.tile([P, P], bf16, tag="sv")
                            nc.vector.tensor_scalar(
                                out=sv[:], in0=iota_t[:],
                                scalar1=srel_t[:, ch:ch + 1],
                                scalar2=val_t[:, ch:ch + 1],
                                op0=mybir.AluOpType.is_equal,
                                op1=mybir.AluOpType.mult,
                            )
                        if ablate == "nope":
                            continue
                        acc = acc_of(gid)
                        nc.tensor.matmul(
                            out=acc[:, 64 * jj:64 * jj + fcols],
                            lhsT=sv[:],
                            rhs=g3[:, t, 0:fcols],
                            start=(ch == st["gb_first"][gid]),
                            stop=(ch == st["gb_last"][gid]),
                            skip_group_check=True,
                        )

            # ---- phase 1: z1 = A@x ; dense chain ; z2 piece shards ----
            ag_issued = [False] * NPIECE

            def piece_of_block(j):
                return j // pb

            def phase1_block(j, acc_ap):
                z1_sb = dns.tile([P, F_IN], bf16, tag="z1")
                nc.vector.tensor_copy(out=z1_sb[:], in_=acc_ap)
                pt = psd.tile([F_IN, P], bf16, tag="pt")
                nc.tensor.transpose(out=pt[:], in_=z1_sb[:], identity=ident_t[:])
                z1t = dns.tile([F_IN, P], bf16, tag="z1t")
                nc.vector.tensor_copy(out=z1t[:], in_=pt[:])
                ph = psd.tile([F_HID, P], f32, tag="pd")
                nc.tensor.matmul(out=ph[:], lhsT=w1_t[:], rhs=z1t[:],
                                 start=True, stop=True)
                ht = dns.tile([F_HID, P], bf16, tag="ht")
                nc.scalar.activation(out=ht[:], in_=ph[:],
                                     func=mybir.ActivationFunctionType.Relu)
                pz = psd.tile([P, 64], f32, tag="pd")
                nc.tensor.matmul(out=pz[:], lhsT=ht[:], rhs=w2_t[:],
                                 start=True, stop=True)
                z2_sb = dns.tile([P, 64], bf16, tag="z2")
                nc.scalar.copy(out=z2_sb[:], in_=pz[:])
                p = piece_of_block(j)
                jl = j - p * pb
                nc.sync.dma_start(
                    out=z2locp[p][jl * P:(jl + 1) * P, 0:64], in_=z2_sb[:])
                if only_phase == 1:
                    o1 = dns.tile([P, 64], f32, tag="o1")
                    nc.vector.tensor_copy(out=o1[:], in_=pz[:])
                    nc.sync.dma_start(
                        out=out_ext[j * P:(j + 1) * P, :], in_=o1[:, 0:F_OUT])

            def issue_ag(p):
                out3 = z2p[p][:].rearrange("(c r) f -> c r f", c=NC)
                nc.gpsimd.collective_compute(
                    "AllGather",
                    mybir.AluOpType.bypass,
                    replica_groups=[list(range(NC))],
                    ins=[z2locp[p][:].opt()],
                    outs=[out3[:, :, :].opt()],
                )

            slabs1 = st1["slabs"]
            slabs3 = st3["slabs"]
            acc1 = {}
            acc3 = {}

            def emit_ph1_sb(S):
                jlo, jhi = S * SUPER, min((S + 1) * SUPER, NBLK)
                acc_t = psum.tile([P, 64 * (jhi - jlo)], f32, tag="acc")
                acc1[S] = acc_t
                do_slabs([s for s in slabs1 if s[3] == S], st1,
                         idx1_t, val1_t, srel1_t,
                         lambda b: xtab[bank_rows(b), :], F_IN,
                         lambda gid: acc1[gid], "g1")
                if ablate in ("gonly", "nope"):
                    return
                if ablate in ("nodense", "svhoist"):
                    dr = dns.tile([P, 64 * (jhi - jlo)], bf16, tag="dr")
                    nc.scalar.copy(out=dr[:], in_=acc_t[:])
                    return
                for j in range(jlo, jhi):
                    jj = j - jlo
                    phase1_block(j, acc_t[:, 64 * jj:64 * jj + F_IN])

            def emit_ph3_piece(pp):
                for S in range(NSB):
                    gid = pp * NSB + S
                    jlo, jhi = S * SUPER, min((S + 1) * SUPER, NBLK)
                    acc_t = psum.tile([P, 64 * (jhi - jlo)], f32, tag="acc")
                    acc3[gid] = acc_t
                    do_slabs([s for s in slabs3 if s[3] == gid], st3,
                             idx3_t, val3_t, srel3_t,
                             lambda b: z2p[b][:, :], F_OUT,
                             lambda g: acc3[g], "g2")
                    for j in range(jlo, jhi):
                        jj = j - jlo
                        src = acc_t[:, 64 * jj:64 * jj + F_OUT]
                        dsts = outacc[:, j * F_OUT:(j + 1) * F_OUT]
                        if pp == 0:
                            nc.scalar.copy(out=dsts, in_=src)
                        else:
                            nc.vector.tensor_tensor(
                                out=dsts, in0=dsts, in1=src,
                                op=mybir.AluOpType.add)

            if only_phase == 3:
                for p in range(NPIECE):
                    nc.sync.dma_start(
                        out=z2locp[p][:, :],
                        in_=z2in[p * pb * P:(p * pb + plens[p]) * P, :])
                    issue_ag(p)
                for pp in range(NPIECE):
                    emit_ph3_piece(pp)
            elif only_phase == 1 or ablate is not None:
                for S in range(NSB):
                    emit_ph1_sb(S)
                    if ablate is None:
                        for p in range(NPIECE):
                            if not ag_issued[p] and \
                               min((S + 1) * SUPER, NBLK) >= min((p + 1) * pb, NBLK):
                                ag_issued[p] = True
                                issue_ag(p)
                if ablate is not None:
                    nc.vector.memset(outacc[:, 0:F_OUT], 0.0)
                    nc.sync.dma_start(out=out_ext[0:P, :],
                                      in_=outacc[:, 0:F_OUT])
            else:
                # full: interleave — AGs issued one SB after data-ready;
                # ph3 pieces emitted once their AG has had time to land.
                def ready_sb(p):
                    tgt = min((p + 1) * pb, NBLK)
                    for S in range(NSB):
                        if min((S + 1) * SUPER, NBLK) >= tgt:
                            return S
                    return NSB - 1

                ag_after = {}
                ph3_after = {}
                for p in range(NPIECE):
                    ag_after.setdefault(min(ready_sb(p) + 1, NSB - 1), []).append(p)
                    ph3_after.setdefault(min(ready_sb(p) + 4, NSB - 1), []).append(p)
                for S in range(NSB):
                    emit_ph1_sb(S)
                    for p in ag_after.get(S, []):
                        issue_ag(p)
                    for pp in ph3_after.get(S, []):
                        emit_ph3_piece(pp)

            if only_phase != 1 and ablate is None:
                # final output DMA (one per block)
                oview = out_ext[:].rearrange("(j s) f -> s j f", s=P)
                nc.sync.dma_start(
                    out=oview[:, :, :],
                    in_=outacc[:].rearrange("s (j f) -> s j f", f=F_OUT))

    nc.compile()
    return nc


def pack_inputs(L, x, W1, W2):
    """Returns per-core in_maps list."""
    TAB = L["TAB"]
    st1, st3 = L["st1"], L["st3"]
    xtab = np.zeros((TAB, FE), ml_dtypes.bfloat16)
    xtab[L["table_row"], 0:F_IN] = x.astype(ml_dtypes.bfloat16)
    w1b = W1.astype(ml_dtypes.bfloat16)
    w2b = np.zeros((F_HID, 64), ml_dtypes.bfloat16)
    w2b[:, 0:F_OUT] = W2.astype(ml_dtypes.bfloat16)
    iota = np.tile(np.arange(P, dtype=np.float32), (P, 1)).astype(ml_dtypes.bfloat16)

    idx1 = wrap_cols(st1["gidx"], st1["NCHUNK"])
    idx3 = wrap_cols(st3["gidx"], st3["NCHUNK"])

    in_maps = []
    for c in range(NC):
        m = {
            "xtab": xtab,
            "gidx1": idx1[c],
            "gval1": st1["gval"][c].reshape(st1["NCHUNK"], P).T.copy(),
            "gsrel1": st1["gsrel"][c].reshape(st1["NCHUNK"], P).T.copy(),
            "gidx3": idx3[c],
            "gval3": st3["gval"][c].reshape(st3["NCHUNK"], P).T.copy(),
            "gsrel3": st3["gsrel"][c].reshape(st3["NCHUNK"], P).T.copy(),
            "w1": w1b, "w2": w2b, "iota": iota,
        }
        in_maps.append(m)
    return in_maps


def unpack_output(L, results):
    """results: list of per-core dicts with 'out' [NBLK*128, 40]."""
    outcat = np.concatenate([r["out"] for r in results], axis=0)  # [TAB, 40]
    return outcat[L["table_row"]]


def make_runner(nc, n_cores=8, donate=False):
    install_neuronx_cc_hook()
    partition_name = nc.partition_id_tensor.name if nc.partition_id_tensor else None

    in_names, out_names, out_avals, zero_outs = [], [], [], []
    for alloc in nc.m.functions[0].allocations:
        if not isinstance(alloc, mybir.MemoryLocationSet):
            continue
        name = alloc.memorylocations[0].name
        if alloc.kind == "ExternalInput":
            if name != partition_name:
                in_names.append(name)
        elif alloc.kind == "ExternalOutput":
            out_names.append(name)
            shape = tuple(alloc.tensor_shape)
            dtype = mybir.dt.np(alloc.dtype)
            out_avals.append(jax.core.ShapedArray(shape, dtype))
            zero_outs.append(np.zeros(shape, dtype))
    n_params = len(in_names)
    n_outs = len(out_avals)
    all_in_names = list(in_names) + list(out_names)
    if partition_name is not None:
        all_in_names.append(partition_name)

    def _body(*args):
        operands = list(args)
        if partition_name is not None:
            operands.append(partition_id_tensor())
        outs = _bass_exec_p.bind(
            *operands,
            out_avals=tuple(out_avals),
            in_names=tuple(all_in_names),
            out_names=tuple(out_names),
            lowering_input_output_aliases=(),
            sim_require_finite=True,
            sim_require_nnan=True,
            nc=nc,
        )
        return tuple(outs)

    devices = jax.devices()[:n_cores]
    mesh = Mesh(np.asarray(devices), ("core",))
    in_specs = (PartitionSpec("core"),) * (n_params + n_outs)
    out_specs = (PartitionSpec("core"),) * n_outs
    jit_kwargs = {"keep_unused": True}
    if donate:
        jit_kwargs["donate_argnums"] = tuple(range(n_params, n_params + n_outs))
    fn = jax.jit(
        shard_map(_body, mesh=mesh, in_specs=in_specs, out_specs=out_specs,
                  check_rep=False),
        **jit_kwargs,
    )
    sharding = NamedSharding(mesh, PartitionSpec("core"))

    class Runner:
        def __init__(self):
            self.fn = fn
            self.in_names = in_names
            self.out_names = out_names
            self.n_cores = n_cores
            self.sharding = sharding
            self.zero_outs = zero_outs

        def put_inputs(self, in_maps):
            args = []
            for name in in_names:
                cat = np.concatenate([np.asarray(m[name]) for m in in_maps], axis=0)
                args.append(jax.device_put(cat, sharding))
            for z in zero_outs:
                cat = np.concatenate([z] * n_cores, axis=0)
                args.append(jax.device_put(cat, sharding))
            return args

        def __call__(self, args):
            return self.fn(*args)

        def run(self, in_maps):
            args = self.put_inputs(in_maps)
            outs = self.fn(*args)
            jax.block_until_ready(outs)
            res = []
            for c in range(n_cores):
                d = {}
                for i, name in enumerate(out_names):
                    arr = np.asarray(outs[i])
                    per = arr.shape[0] // n_cores
                    d[name] = arr[c * per:(c + 1) * per]
                res.append(d)
            return res

    return Runner()


_CACHE = {}


def kernel(src, dst, vals, x, W1, W2):
    src = np.asarray(src); dst = np.asarray(dst)
    vals = np.asarray(vals, dtype=np.float32)
    x = np.asarray(x, dtype=np.float32)
    W1 = np.asarray(W1, dtype=np.float32)
    W2 = np.asarray(W2, dtype=np.float32)

    L = build_layout(src.astype(np.int64), dst.astype(np.int64), vals, NUM_NODES)
    key = "r"
    if key not in _CACHE:
        nc = build_nc(L)
        _CACHE[key] = make_runner(nc)
    r = _CACHE[key]
    in_maps = pack_inputs(L, x, W1, W2)
    results = r.run(in_maps)
    return unpack_output(L, results).astype(np.float32)


# revision 29
# speedup vs baseline: 1.4385x; 1.2194x over previous
"""Self-contained Trainium2 Bass kernel for the 2-layer GCN problem.

kernel(src, dst, vals, x, W1, W2) -> [80000, 40] float32 logits,
computed as  A @ (relu((A @ x) @ W1) @ W2)  on 8 NeuronCores.

Strategy: nodes sharded round-robin across cores in 128-node slots
(graph-parallel); W1/W2 replicated.  SpMM = banked int16 dma_gather of
256B table rows (4 SWDGE queues) + per-chunk selection-matrix matmuls
accumulated in PSUM.  The cross-partition z2 exchange is 4 piecewise
AllGathers into Shared DRAM tensors, issued as phase-1 superblocks
complete so phase-3 gathers overlap the tail of phase 1.
"""
import numpy as np
import ml_dtypes
import jax
from jax.sharding import Mesh, PartitionSpec, NamedSharding
from jax.experimental.shard_map import shard_map

import concourse.bass as bass
import concourse.bacc as bacc
import concourse.tile as tile
import concourse.mybir as mybir
from concourse import bass2jax
from concourse.bass2jax import _bass_exec_p, install_neuronx_cc_hook, partition_id_tensor
from concourse.masks import make_identity

NUM_NODES = 80000
NUM_EDGES = 1280000

import os
NC, P, GRP = 8, 128, 1024
SUPER = 8
SLABMAX = int(os.environ.get("SLABMAX", 48))
G3BUFS = int(os.environ.get("G3BUFS", 4))
NPIECE = int(os.environ.get("NPIECE", 3))

F_IN = 64      # x features
F_HID = 128
F_OUT = 40
FE = 128       # bf16 elements per 256B table row

bf16 = mybir.dt.bfloat16
f32 = mybir.dt.float32


def build_stream(ec, ej, bank, lidx, vals, srel, NBLK, NB, order,
                 guard="block"):
    """Pack an edge list into the banked chunk stream.

    ec/ej: src core/block per edge; bank/lidx: gather bank + row-in-bank
    per edge; order: "Sb" (superblock outer, bank inner — phase 1) or
    "bS" (bank/piece outer — phase 3).  Returns chunk stream dict.
    """
    E = ec.shape[0]
    NSB = -(-NBLK // SUPER)
    lidx = lidx.astype(np.int16)

    key = (ec * NBLK + ej) * NB + bank
    ord_e = np.lexsort((lidx, key))
    ks = key[ord_e]
    first = np.r_[0, np.flatnonzero(np.diff(ks)) + 1]
    group_start = np.zeros(E, np.int64)
    group_start[first] = first
    group_start = np.maximum.accumulate(group_start)
    k_in_group = np.arange(E) - group_start

    cnt = np.bincount(key, minlength=NC * NBLK * NB).reshape(NC, NBLK, NB)
    Kc = -(-cnt // P)
    Kg = Kc.max(axis=0)                       # [NBLK, NB]
    if guard == "block":
        empty = Kg.sum(axis=1) == 0
        Kg[empty, 0] = 1
    else:                                     # per-cell guard (phase 3)
        Kg = np.maximum(Kg, 1)

    Kmax = int(Kg.max())
    # chunk_map[j, b, k] -> chunk id of the k-th chunk of cell (j, b)
    chunk_map = np.full((NBLK, NB, Kmax), -1, np.int64)
    slabs = []            # (start_chunk, n_chunks, bank, group_id)
    chunk_block = []
    gb_first = {}
    gb_last = {}
    pos = 0

    def emit_run(js, b, gid):
        """Emit cell (j in js, bank b) chunks interleaved round-robin over j
        so consecutive matmuls hit different PSUM regions."""
        nonlocal pos
        run_start = pos
        items = [(k, j) for j in js for k in range(Kg[j, b])]
        items.sort()
        for (k, j) in items:
            chunk_map[j, b, k] = pos
            chunk_block.append(j)
            pos += 1
        run_len = pos - run_start
        o = run_start
        while run_len > 0:
            take = min(run_len, SLABMAX)
            slabs.append((o, take, b, gid))
            o += take
            run_len -= take

    if order == "Sb":
        for S in range(NSB):
            js = list(range(S * SUPER, min((S + 1) * SUPER, NBLK)))
            gid = S
            g0 = pos
            for b in range(NB):
                emit_run(js, b, gid)
            gb_first[gid] = g0
            gb_last[gid] = pos - 1
    else:  # "bS": bank (piece) outer, superblock inner
        for b in range(NB):
            for S in range(NSB):
                js = list(range(S * SUPER, min((S + 1) * SUPER, NBLK)))
                gid = b * NSB + S
                g0 = pos
                emit_run(js, b, gid)
                gb_first[gid] = g0
                gb_last[gid] = pos - 1
    NCHUNK = pos
    chunk_block = np.asarray(chunk_block)

    gidx = np.zeros((NC, NCHUNK * P), np.int16)
    gval = np.zeros((NC, NCHUNK * P), np.float32)
    gsrel = np.zeros((NC, NCHUNK * P), np.float32)
    echunk = chunk_map[ej[ord_e], bank[ord_e], k_in_group // P]
    assert (echunk >= 0).all()
    epos = echunk * P + (k_in_group % P)
    core_o = ec[ord_e]
    gidx[core_o, epos] = lidx[ord_e]
    gval[core_o, epos] = vals[ord_e]
    gsrel[core_o, epos] = srel[ord_e]

    return dict(NCHUNK=NCHUNK, slabs=slabs, chunk_block=chunk_block,
                gb_first=gb_first, gb_last=gb_last, Kg=Kg,
                gidx=gidx, gval=gval, gsrel=gsrel)


def build_layout(src, dst, vals, n_nodes, banksz=32768):
    NBLK = -(-n_nodes // GRP)
    TAB = NC * NBLK * P
    NB1 = -(-TAB // banksz)
    NSB = -(-NBLK // SUPER)

    # LPT balance: assign nodes to (core, block) groups by out-degree so
    # per-group edge totals (and hence per-core chunk counts) equalize.
    import heapq
    deg = np.bincount(src, minlength=n_nodes)
    order = np.argsort(-deg, kind="stable")
    ngroups = NC * NBLK
    cap = n_nodes - (ngroups - 1) * P  # last-filled groups may be partial
    heap = [(0, g) for g in range(ngroups)]
    heapq.heapify(heap)
    counts = np.zeros(ngroups, np.int64)
    c_of = np.empty(n_nodes, np.int64)
    j_of = np.empty(n_nodes, np.int64)
    s_of = np.empty(n_nodes, np.int64)
    for node in order:
        load, g = heapq.heappop(heap)
        c_of[node] = g % NC
        j_of[node] = g // NC
        s_of[node] = counts[g]
        counts[g] += 1
        if counts[g] < P:
            heapq.heappush(heap, (load + int(deg[node]), g))
    table_row = (c_of * NBLK + j_of) * P + s_of

    ec, ej, es = c_of[src], j_of[src], s_of[src]
    srel = es.astype(np.float32)

    # phase 1: dst -> xtab row / bank
    r1 = table_row[dst]
    b1 = r1 // banksz
    l1 = r1 % banksz
    st1 = build_stream(ec, ej, b1, l1, vals, srel, NBLK, NB1, "Sb",
                       guard="block")

    # phase 3: dst -> piece (block range of its owner), piece-local row
    pb = -(-NBLK // NPIECE)
    plens = [min((p + 1) * pb, NBLK) - p * pb for p in range(NPIECE)]
    pjd = j_of[dst]
    pcd = c_of[dst]
    psd_ = s_of[dst]
    p3 = pjd // pb
    assert NC * pb * P <= 32768, "piece rows exceed int16 gather index range"
    l3 = (pcd * np.asarray(plens)[p3] + (pjd - p3 * pb)) * P + psd_
    st3 = build_stream(ec, ej, p3, l3, vals, srel, NBLK, NPIECE, "bS",
                       guard="cell")

    return dict(NBLK=NBLK, TAB=TAB, NB1=NB1, NSB=NSB, banksz=banksz,
                pb=pb, plens=plens, table_row=table_row, st1=st1, st3=st3)


def wrap_cols(a, NCHUNK):
    """[NC, NCHUNK*128] -> per-core [128, NCHUNK*8] wrapped int16 tiles."""
    out = []
    for c in range(NC):
        n = a.shape[1]
        w = a[c].reshape(n // 16, 16).T
        out.append(np.tile(w, (8, 1)).copy())
    return out


def build_nc(L, shared_out=True, only_phase=None, ablate=None):
    NBLK, TAB, NB1, NSB = L["NBLK"], L["TAB"], L["NB1"], L["NSB"]
    banksz = L["banksz"]
    pb, plens = L["pb"], L["plens"]
    st1, st3 = L["st1"], L["st3"]
    NCH1, NCH3 = st1["NCHUNK"], st3["NCHUNK"]
    maxslab = max(s[1] for s in st1["slabs"] + st3["slabs"])

    nc = bacc.Bacc("TRN2", target_bir_lowering=False, debug=False,
                   num_devices=NC, num_swdge_queues=4)
    xtab = nc.dram_tensor("xtab", [TAB, FE], bf16, kind="ExternalInput")
    z2in = None
    if only_phase == 3:
        z2in = nc.dram_tensor("z2in", [NBLK * P, FE], bf16, kind="ExternalInput")
    gidx1 = nc.dram_tensor("gidx1", [P, NCH1 * 8], mybir.dt.int16, kind="ExternalInput")
    gval1 = nc.dram_tensor("gval1", [P, NCH1], f32, kind="ExternalInput")
    gsrel1 = nc.dram_tensor("gsrel1", [P, NCH1], f32, kind="ExternalInput")
    gidx3 = nc.dram_tensor("gidx3", [P, NCH3 * 8], mybir.dt.int16, kind="ExternalInput")
    gval3 = nc.dram_tensor("gval3", [P, NCH3], f32, kind="ExternalInput")
    gsrel3 = nc.dram_tensor("gsrel3", [P, NCH3], f32, kind="ExternalInput")
    w1 = nc.dram_tensor("w1", [F_IN, F_HID], bf16, kind="ExternalInput")
    w2 = nc.dram_tensor("w2", [F_HID, 64], bf16, kind="ExternalInput")
    iota_in = nc.dram_tensor("iota", [P, P], bf16, kind="ExternalInput")
    out_ext = nc.dram_tensor("out", [NBLK * P, F_OUT], f32, kind="ExternalOutput")

    def bank_rows(b):
        return slice(b * banksz, min((b + 1) * banksz, TAB))

    with tile.TileContext(nc) as tc:
        with (
            tc.tile_pool(name="cons", bufs=1) as cons,
            tc.tile_pool(name="sbuf", bufs=G3BUFS) as sbuf,
            tc.tile_pool(name="sv", bufs=32) as svp,
            tc.tile_pool(name="dense", bufs=2) as dns,
            tc.tile_pool(name="psum", bufs=2, space="PSUM") as psum,
            tc.tile_pool(name="psd", bufs=2, space="PSUM") as psd,
            tc.tile_pool(name="dram", bufs=1, space="DRAM") as dram,
        ):
            iota_t = cons.tile([P, P], bf16)
            ident_t = cons.tile([P, P], bf16)
            make_identity(nc, ident_t[:])
            w1_t = cons.tile([F_IN, F_HID], bf16)
            w2_t = cons.tile([F_HID, 64], bf16)
            idx1_t = cons.tile([P, NCH1 * 8], mybir.dt.int16)
            val1_t = cons.tile([P, NCH1], f32)
            srel1_t = cons.tile([P, NCH1], f32)
            idx3_t = cons.tile([P, NCH3 * 8], mybir.dt.int16)
            val3_t = cons.tile([P, NCH3], f32)
            srel3_t = cons.tile([P, NCH3], f32)
            outacc = cons.tile([P, NBLK * F_OUT], f32)
            nc.sync.dma_start(out=iota_t[:], in_=iota_in[:, :])
            nc.sync.dma_start(out=w1_t[:], in_=w1[:, :])
            nc.sync.dma_start(out=w2_t[:], in_=w2[:, :])
            nc.sync.dma_start(out=idx1_t[:], in_=gidx1[:, :])
            nc.sync.dma_start(out=val1_t[:], in_=gval1[:, :])
            nc.sync.dma_start(out=srel1_t[:], in_=gsrel1[:, :])
            nc.sync.dma_start(out=idx3_t[:], in_=gidx3[:, :])
            nc.sync.dma_start(out=val3_t[:], in_=gval3[:, :])
            nc.sync.dma_start(out=srel3_t[:], in_=gsrel3[:, :])

            z2locp = [dram.tile([plens[p] * P, FE], bf16, name=f"z2locp{p}")
                      for p in range(NPIECE)]
            z2p = [dram.tile([NC * plens[p] * P, FE], bf16, name=f"z2p{p}",
                             addr_space=("Shared" if shared_out else "Local"))
                   for p in range(NPIECE)]

            qctr = [0]
            sv_hoist = None
            if ablate == "svhoist":
                sv_hoist = cons.tile([P, P], bf16)
                nc.vector.tensor_scalar(
                    out=sv_hoist[:], in0=iota_t[:],
                    scalar1=srel1_t[:, 0:1], scalar2=val1_t[:, 0:1],
                    op0=mybir.AluOpType.is_equal, op1=mybir.AluOpType.mult)

            def do_slabs(slabs_sel, st, idx_t, val_t, srel_t, table_of,
                         fcols, acc_of, gtag):
                """Run gather+selection-matmul for the given slab list."""
                for (c0, Ln, b, gid) in slabs_sel:
                    g3 = sbuf.tile([P, maxslab, FE], bf16, tag=gtag)
                    nc.gpsimd.dma_gather(
                        g3[:, 0:Ln, :],
                        table_of(b),
                        idx_t[:, c0 * 8:(c0 + Ln) * 8],
                        Ln * P,
                        Ln * P,
                        FE,
                        single_packet=False,
                        queue_num=(qctr[0] % 4),
                    )
                    qctr[0] += 1
                    if ablate == "gonly":
                        continue
                    for t in range(Ln):
                        ch = c0 + t
                        j = int(st["chunk_block"][ch])
                        jj = j % SUPER
                        if ablate == "svhoist":
                            sv = sv_hoist
                        else:
                            sv = svp.tile([P, P], bf16, tag="sv")
                            nc.vector.tensor_scalar(
                                out=sv[:], in0=iota_t[:],
                                scalar1=srel_t[:, ch:ch + 1],
                                scalar2=val_t[:, ch:ch + 1],
                                op0=mybir.AluOpType.is_equal,
                                op1=mybir.AluOpType.mult,
                            )
                        if ablate == "nope":
                            continue
                        acc = acc_of(gid)
                        nc.tensor.matmul(
                            out=acc[:, 64 * jj:64 * jj + fcols],
                            lhsT=sv[:],
                            rhs=g3[:, t, 0:fcols],
                            start=(ch == st["gb_first"][gid]),
                            stop=(ch == st["gb_last"][gid]),
                            skip_group_check=True,
                        )

            # ---- phase 1: z1 = A@x ; dense chain ; z2 piece shards ----
            ag_issued = [False] * NPIECE

            def piece_of_block(j):
                return j // pb

            def phase1_block(j, acc_ap):
                z1_sb = dns.tile([P, F_IN], bf16, tag="z1")
                nc.vector.tensor_copy(out=z1_sb[:], in_=acc_ap)
                pt = psd.tile([F_IN, P], bf16, tag="pt")
                nc.tensor.transpose(out=pt[:], in_=z1_sb[:], identity=ident_t[:])
                z1t = dns.tile([F_IN, P], bf16, tag="z1t")
                nc.vector.tensor_copy(out=z1t[:], in_=pt[:])
                ph = psd.tile([F_HID, P], f32, tag="pd")
                nc.tensor.matmul(out=ph[:], lhsT=w1_t[:], rhs=z1t[:],
                                 start=True, stop=True)
                ht = dns.tile([F_HID, P], bf16, tag="ht")
                nc.scalar.activation(out=ht[:], in_=ph[:],
                                     func=mybir.ActivationFunctionType.Relu)
                pz = psd.tile([P, 64], f32, tag="pd")
                nc.tensor.matmul(out=pz[:], lhsT=ht[:], rhs=w2_t[:],
                                 start=True, stop=True)
                z2_sb = dns.tile([P, 64], bf16, tag="z2")
                nc.scalar.copy(out=z2_sb[:], in_=pz[:])
                p = piece_of_block(j)
                jl = j - p * pb
                nc.sync.dma_start(
                    out=z2locp[p][jl * P:(jl + 1) * P, 0:64], in_=z2_sb[:])
                if only_phase == 1:
                    o1 = dns.tile([P, 64], f32, tag="o1")
                    nc.vector.tensor_copy(out=o1[:], in_=pz[:])
                    nc.sync.dma_start(
                        out=out_ext[j * P:(j + 1) * P, :], in_=o1[:, 0:F_OUT])

            def issue_ag(p):
                out3 = z2p[p][:].rearrange("(c r) f -> c r f", c=NC)
                nc.gpsimd.collective_compute(
                    "AllGather",
                    mybir.AluOpType.bypass,
                    replica_groups=[list(range(NC))],
                    ins=[z2locp[p][:].opt()],
                    outs=[out3[:, :, :].opt()],
                )

            slabs1 = st1["slabs"]
            slabs3 = st3["slabs"]
            acc1 = {}
            acc3 = {}

            def emit_ph1_sb(S):
                jlo, jhi = S * SUPER, min((S + 1) * SUPER, NBLK)
                acc_t = psum.tile([P, 64 * (jhi - jlo)], f32, tag="acc")
                acc1[S] = acc_t
                do_slabs([s for s in slabs1 if s[3] == S], st1,
                         idx1_t, val1_t, srel1_t,
                         lambda b: xtab[bank_rows(b), :], F_IN,
                         lambda gid: acc1[gid], "g1")
                if ablate in ("gonly", "nope"):
                    return
                if ablate in ("nodense", "svhoist"):
                    dr = dns.tile([P, 64 * (jhi - jlo)], bf16, tag="dr")
                    nc.scalar.copy(out=dr[:], in_=acc_t[:])
                    return
                for j in range(jlo, jhi):
                    jj = j - jlo
                    phase1_block(j, acc_t[:, 64 * jj:64 * jj + F_IN])

            def emit_ph3_piece(pp):
                for S in range(NSB):
                    gid = pp * NSB + S
                    jlo, jhi = S * SUPER, min((S + 1) * SUPER, NBLK)
                    acc_t = psum.tile([P, 64 * (jhi - jlo)], f32, tag="acc")
                    acc3[gid] = acc_t
                    do_slabs([s for s in slabs3 if s[3] == gid], st3,
                             idx3_t, val3_t, srel3_t,
                             lambda b: z2p[b][:, :], F_OUT,
                             lambda g: acc3[g], "g2")
                    for j in range(jlo, jhi):
                        jj = j - jlo
                        src = acc_t[:, 64 * jj:64 * jj + F_OUT]
                        dsts = outacc[:, j * F_OUT:(j + 1) * F_OUT]
                        if pp == 0:
                            nc.scalar.copy(out=dsts, in_=src)
                        else:
                            nc.vector.tensor_tensor(
                                out=dsts, in0=dsts, in1=src,
                                op=mybir.AluOpType.add)

            if only_phase == 3:
                for p in range(NPIECE):
                    nc.sync.dma_start(
                        out=z2locp[p][:, :],
                        in_=z2in[p * pb * P:(p * pb + plens[p]) * P, :])
                    issue_ag(p)
                for pp in range(NPIECE):
                    emit_ph3_piece(pp)
            elif only_phase == 1 or ablate is not None:
                for S in range(NSB):
                    emit_ph1_sb(S)
                    if ablate is None:
                        for p in range(NPIECE):
                            if not ag_issued[p] and \
                               min((S + 1) * SUPER, NBLK) >= min((p + 1) * pb, NBLK):
                                ag_issued[p] = True
                                issue_ag(p)
                if ablate is not None:
                    nc.vector.memset(outacc[:, 0:F_OUT], 0.0)
                    nc.sync.dma_start(out=out_ext[0:P, :],
                                      in_=outacc[:, 0:F_OUT])
            else:
                # full: interleave — AGs issued one SB after data-ready;
                # ph3 pieces emitted once their AG has had time to land.
                def ready_sb(p):
                    tgt = min((p + 1) * pb, NBLK)
                    for S in range(NSB):
                        if min((S + 1) * SUPER, NBLK) >= tgt:
                            return S
                    return NSB - 1

                ag_after = {}
                ph3_after = {}
                for p in range(NPIECE):
                    ag_after.setdefault(min(ready_sb(p) + 1, NSB - 1), []).append(p)
                    ph3_after.setdefault(min(ready_sb(p) + 3, NSB - 1), []).append(p)
                for S in range(NSB):
                    emit_ph1_sb(S)
                    for p in ag_after.get(S, []):
                        issue_ag(p)
                    for pp in ph3_after.get(S, []):
                        emit_ph3_piece(pp)

            if only_phase != 1 and ablate is None:
                # final output DMA (one per block)
                oview = out_ext[:].rearrange("(j s) f -> s j f", s=P)
                nc.sync.dma_start(
                    out=oview[:, :, :],
                    in_=outacc[:].rearrange("s (j f) -> s j f", f=F_OUT))

    nc.compile()
    return nc


def pack_inputs(L, x, W1, W2):
    """Returns per-core in_maps list."""
    TAB = L["TAB"]
    st1, st3 = L["st1"], L["st3"]
    xtab = np.zeros((TAB, FE), ml_dtypes.bfloat16)
    xtab[L["table_row"], 0:F_IN] = x.astype(ml_dtypes.bfloat16)
    w1b = W1.astype(ml_dtypes.bfloat16)
    w2b = np.zeros((F_HID, 64), ml_dtypes.bfloat16)
    w2b[:, 0:F_OUT] = W2.astype(ml_dtypes.bfloat16)
    iota = np.tile(np.arange(P, dtype=np.float32), (P, 1)).astype(ml_dtypes.bfloat16)

    idx1 = wrap_cols(st1["gidx"], st1["NCHUNK"])
    idx3 = wrap_cols(st3["gidx"], st3["NCHUNK"])

    in_maps = []
    for c in range(NC):
        m = {
            "xtab": xtab,
            "gidx1": idx1[c],
            "gval1": st1["gval"][c].reshape(st1["NCHUNK"], P).T.copy(),
            "gsrel1": st1["gsrel"][c].reshape(st1["NCHUNK"], P).T.copy(),
            "gidx3": idx3[c],
            "gval3": st3["gval"][c].reshape(st3["NCHUNK"], P).T.copy(),
            "gsrel3": st3["gsrel"][c].reshape(st3["NCHUNK"], P).T.copy(),
            "w1": w1b, "w2": w2b, "iota": iota,
        }
        in_maps.append(m)
    return in_maps


def unpack_output(L, results):
    """results: list of per-core dicts with 'out' [NBLK*128, 40]."""
    outcat = np.concatenate([r["out"] for r in results], axis=0)  # [TAB, 40]
    return outcat[L["table_row"]]


def make_runner(nc, n_cores=8, donate=False):
    install_neuronx_cc_hook()
    partition_name = nc.partition_id_tensor.name if nc.partition_id_tensor else None

    in_names, out_names, out_avals, zero_outs = [], [], [], []
    for alloc in nc.m.functions[0].allocations:
        if not isinstance(alloc, mybir.MemoryLocationSet):
            continue
        name = alloc.memorylocations[0].name
        if alloc.kind == "ExternalInput":
            if name != partition_name:
                in_names.append(name)
        elif alloc.kind == "ExternalOutput":
            out_names.append(name)
            shape = tuple(alloc.tensor_shape)
            dtype = mybir.dt.np(alloc.dtype)
            out_avals.append(jax.core.ShapedArray(shape, dtype))
            zero_outs.append(np.zeros(shape, dtype))
    n_params = len(in_names)
    n_outs = len(out_avals)
    all_in_names = list(in_names) + list(out_names)
    if partition_name is not None:
        all_in_names.append(partition_name)

    def _body(*args):
        operands = list(args)
        if partition_name is not None:
            operands.append(partition_id_tensor())
        outs = _bass_exec_p.bind(
            *operands,
            out_avals=tuple(out_avals),
            in_names=tuple(all_in_names),
            out_names=tuple(out_names),
            lowering_input_output_aliases=(),
            sim_require_finite=True,
            sim_require_nnan=True,
            nc=nc,
        )
        return tuple(outs)

    devices = jax.devices()[:n_cores]
    mesh = Mesh(np.asarray(devices), ("core",))
    in_specs = (PartitionSpec("core"),) * (n_params + n_outs)
    out_specs = (PartitionSpec("core"),) * n_outs
    jit_kwargs = {"keep_unused": True}
    if donate:
        jit_kwargs["donate_argnums"] = tuple(range(n_params, n_params + n_outs))
    fn = jax.jit(
        shard_map(_body, mesh=mesh, in_specs=in_specs, out_specs=out_specs,
                  check_rep=False),
        **jit_kwargs,
    )
    sharding = NamedSharding(mesh, PartitionSpec("core"))

    class Runner:
        def __init__(self):
            self.fn = fn
            self.in_names = in_names
            self.out_names = out_names
            self.n_cores = n_cores
            self.sharding = sharding
            self.zero_outs = zero_outs

        def put_inputs(self, in_maps):
            args = []
            for name in in_names:
                cat = np.concatenate([np.asarray(m[name]) for m in in_maps], axis=0)
                args.append(jax.device_put(cat, sharding))
            for z in zero_outs:
                cat = np.concatenate([z] * n_cores, axis=0)
                args.append(jax.device_put(cat, sharding))
            return args

        def __call__(self, args):
            return self.fn(*args)

        def run(self, in_maps):
            args = self.put_inputs(in_maps)
            outs = self.fn(*args)
            jax.block_until_ready(outs)
            res = []
            for c in range(n_cores):
                d = {}
                for i, name in enumerate(out_names):
                    arr = np.asarray(outs[i])
                    per = arr.shape[0] // n_cores
                    d[name] = arr[c * per:(c + 1) * per]
                res.append(d)
            return res

    return Runner()


_CACHE = {}


def kernel(src, dst, vals, x, W1, W2):
    src = np.asarray(src); dst = np.asarray(dst)
    vals = np.asarray(vals, dtype=np.float32)
    x = np.asarray(x, dtype=np.float32)
    W1 = np.asarray(W1, dtype=np.float32)
    W2 = np.asarray(W2, dtype=np.float32)

    L = build_layout(src.astype(np.int64), dst.astype(np.int64), vals, NUM_NODES)
    key = "r"
    if key not in _CACHE:
        nc = build_nc(L)
        _CACHE[key] = make_runner(nc)
    r = _CACHE[key]
    in_maps = pack_inputs(L, x, W1, W2)
    results = r.run(in_maps)
    return unpack_output(L, results).astype(np.float32)
